# revision 2
# baseline (speedup 1.0000x reference)
"""ExpertLinear (MoE routing) Trainium2 Bass kernel — fp8 DoubleRow version.

y[b,:] = sum_k ew[b,k] * (x[b,:] @ W[k].T) + (ew @ bias)[b,:]

Strategy: 8-way data-parallel over the batch B across the 8 NeuronCores.
Per core (B_loc = 1024) the blended-expert matmul runs on the PE array in
fp8-e4m3 DoubleRow mode (2 contraction k-tiles per matmul, 0.5 cycles per
output column — 4x the fp32r/bf16 MAC rate). Precision is recovered with a
3-term Karatsuba split, fused into one 24-k-tile PSUM accumulation:

    x@W ~= x1@W1 + xr1@W1 + x1@Wr1        (xr1Wr1 cross term ~1e-3, dropped)

where x1 = e4m3(x*SX), xr1 = e4m3(x*SX - x1), W1 = e4m3(W*SW),
Wr1 = e4m3(W*SW - W1). All four digits share the same power-of-2 scales, so
the three terms accumulate in one PSUM group; 1/(SX*SW) is folded into the
per-partition routing scalars. Measured rel err ~1.2e-3 (vs 2e-2 budget).

Host-side prep supplies per-core:
  xq  [128, nbt, 16, 128]  e4m3: dim2 kt 0..7 = x1 i-tiles, 8..15 = xr1
  wq  [K, 128, 16, OUT]    e4m3: dim2 kt 0..7 = W1 i-tiles, 8..15 = Wr1
  ewp [128, nbt, K]        fp32: routing scalars * 1/(SX*SW)
  ewt [K, B_loc], bias [K, OUT] fp32r: bias seed path (unscaled)

Per (expert, batch-tile): 12 DoubleRow pairs x 2 PSUM half-banks accumulate,
then ACT applies the per-partition routing scale and DVE adds into y_acc.
The second matmul of each (oh0, oh1) pair reuses the stationary operand
(ldweights=False) via the post-compile reuse pass.
"""

import numpy as np
import ml_dtypes

from concourse import bacc
import concourse.mybir as mybir
import concourse.tile as tile
from concourse.bass_utils import run_bass_kernel_spmd

N_CORES = 8
B, K, OUT, IN = 8192, 8, 1024, 1024
P = 128

MM_DT = mybir.dt.float8e4          # e4m3, DoubleRow-capable
E4NP = ml_dtypes.float8_e4m3       # numpy dtype for host-side quantization
SX = 32.0                          # |x| < 5.5 -> |x*SX| < 176 < 240 (e4m3 max)
SW = 65536.0                       # |W| < 2.4e-3 -> |W*SW| < 157 < 240
DR = mybir.MatmulPerfMode.DoubleRow


def build_nc(b_loc=B // N_CORES, k=K, out_dim=OUT, in_dim=IN, mm_dt=MM_DT, rep=1,
             with_bias=True):
    nbt = b_loc // P      # batch tiles per core
    ni = in_dim // P      # contraction i-tiles (per digit)
    nkt = 2 * ni          # fp8 k-tiles per tensor (digit0 0..ni-1, digit1 ni..)
    npair = 3 * ni // 2   # DoubleRow pairs per (expert, batch-tile)
    oh_sz = 512           # PSUM bank = 512 fp32
    noh = out_dim // oh_sz

    nc = bacc.Bacc()
    xq_d = nc.dram_tensor("xq", [P, nbt, nkt, P], mm_dt, kind="ExternalInput")
    wq_d = nc.dram_tensor("wq", [k, P, nkt, out_dim], mm_dt, kind="ExternalInput")
    ewp_d = nc.dram_tensor("ewp", [P, nbt, k], mybir.dt.float32, kind="ExternalInput")
    ewt_d = nc.dram_tensor("ewt", [k, b_loc], mybir.dt.float32r, kind="ExternalInput")
    bias_d = nc.dram_tensor("bias", [k, out_dim], mybir.dt.float32r, kind="ExternalInput")
    y_d = nc.dram_tensor("y", [b_loc, out_dim], mybir.dt.float32, kind="ExternalOutput")

    # (x-ktile-pair, w-ktile-pair) per DoubleRow matmul, in accumulation
    # order: term1 x1@W1, term2 xr1@W1, term3 x1@Wr1
    pair_seq = (
        [(2 * p, 2 * p) for p in range(ni // 2)]
        + [(ni + 2 * p, 2 * p) for p in range(ni // 2)]
        + [(2 * p, ni + 2 * p) for p in range(ni // 2)]
    )
    assert len(pair_seq) == npair

    with tile.TileContext(nc) as tc:
        with (
            tc.tile_pool(name="consts", bufs=1) as consts,
            tc.tile_pool(name="xq", bufs=1) as xq_pool,
            tc.tile_pool(name="yacc", bufs=1) as yacc_pool,
            tc.tile_pool(name="wbuf", bufs=2) as w_pool,
            tc.tile_pool(name="tmp", bufs=4) as tmp_pool,
            tc.tile_pool(name="ps_mm", bufs=4, space="PSUM") as ps_mm_pool,
        ):
            ewp_sb = consts.tile([P, nbt, k], mybir.dt.float32)
            nc.sync.dma_start(ewp_sb[:], ewp_d[:])
            ewt_sb = consts.tile([k, b_loc], mybir.dt.float32r)
            nc.sync.dma_start(ewt_sb[:], ewt_d[:])
            bias_sb = consts.tile([k, out_dim], mybir.dt.float32r)
            nc.sync.dma_start(bias_sb[:], bias_d[:])

            # x digits resident, one tile per batch-tile so the first matmuls
            # only wait for their own slice
            def load_xq(bt):
                xqbt = xq_pool.tile([P, nkt, P], mm_dt, name=f"xq{bt}", tag=f"xq{bt}")
                nc.sync.dma_start(xqbt[:], xq_d[:, bt])
                return xqbt

            def load_w(kk):
                # W digits streamed per-i chunks so matmuls start as soon as
                # the first contraction slices land; chunk c holds k-tiles
                # [2c, 2c+1] of W1 and [ni+2c, ni+2c+1] of Wr1
                wchunks = []
                for c in range(ni // 2):
                    wc = w_pool.tile([P, 2, out_dim], mm_dt, name=f"w1c{c}", tag=f"w1c{c}")
                    nc.sync.dma_start(wc[:], wq_d[kk, :, 2 * c:2 * c + 2, :])
                    wchunks.append(wc)
                for c in range(ni // 2):
                    wc = w_pool.tile([P, 2, out_dim], mm_dt, name=f"wrc{c}", tag=f"wrc{c}")
                    nc.sync.dma_start(wc[:], wq_d[kk, :, ni + 2 * c:ni + 2 * c + 2, :])
                    wchunks.append(wc)
                return wchunks

            xqs = [None] * nbt
            xqs[0] = load_xq(0)
            wchunks_k0 = load_w(0)
            for bt in range(1, nbt):
                xqs[bt] = load_xq(bt)

            y_acc = yacc_pool.tile([P, nbt, out_dim], mybir.dt.float32)

            for _rep in range(rep):
                if with_bias:
                    for bt in range(nbt):
                        pbias = ps_mm_pool.tile(
                            [P, noh, oh_sz], mybir.dt.float32,
                            name="pbias", tag="ps_mm",
                        )
                        for oh in range(noh):
                            nc.tensor.matmul(
                                pbias[:, oh, :],
                                ewt_sb[:, bt * P:(bt + 1) * P],
                                bias_sb[:, oh * oh_sz:(oh + 1) * oh_sz],
                                start=True,
                                stop=True,
                            )
                        for oh in range(noh):
                            nc.scalar.copy(
                                y_acc[:, bt, oh * oh_sz:(oh + 1) * oh_sz],
                                pbias[:, oh, :],
                            )

                for kk in range(k):
                    if kk == 0 and _rep == 0:
                        wchunks = wchunks_k0
                    else:
                        wchunks = load_w(kk)
                    for bt in range(nbt):
                        pss = ps_mm_pool.tile(
                            [P, noh, oh_sz], mybir.dt.float32,
                            name="psmm", tag="ps_mm",
                        )
                        for pi, (xp, wp) in enumerate(pair_seq):
                            lhsT = xqs[bt][:, xp:xp + 2, :]
                            wc = wchunks[wp // 2]
                            for oh in range(noh):
                                nc.tensor.matmul(
                                    pss[:, oh, :],
                                    lhsT,
                                    wc[:, :, oh * oh_sz:(oh + 1) * oh_sz],
                                    start=(pi == 0),
                                    stop=(pi == npair - 1),
                                    perf_mode=DR,
                                )
                        for oh in range(noh):
                            osl = y_acc[:, bt, oh * oh_sz:(oh + 1) * oh_sz]
                            scale = ewp_sb[:, bt, kk:kk + 1]
                            if not with_bias and kk == 0:
                                nc.scalar.mul(osl, pss[:, oh, :], scale)
                            else:
                                tmp = tmp_pool.tile([P, oh_sz], mybir.dt.float32)
                                nc.scalar.mul(tmp[:], pss[:, oh, :], scale)
                                nc.vector.tensor_add(osl, osl, tmp[:])
                        if kk == k - 1:
                            nc.sync.dma_start(
                                y_d[bt * P:(bt + 1) * P, :], y_acc[:, bt, :]
                            )

    nc.compile()

    # Post-compile weight-reuse pass: in the FINAL instruction order, any
    # matmul whose directly-preceding matmul on the PE queue loads the
    # identical stationary AP can skip its reload.
    for blk in nc.m.functions[0].blocks:
        prev_mm = None
        for inst in blk.instructions:
            if isinstance(inst, mybir.InstMatmult):
                if (
                    prev_mm is not None
                    and not inst.is_transpose
                    and not prev_mm.is_transpose
                    and str(prev_mm.ins[1]) == str(inst.ins[1])
                    and prev_mm.tile_position == inst.tile_position
                    and prev_mm.perf_mode == inst.perf_mode
                ):
                    inst.ldweights = False
                prev_mm = inst
    return nc


_NC_CACHE = {}


def _get_nc(with_bias=True):
    key = ("fp8", with_bias)
    if key not in _NC_CACHE:
        _NC_CACHE[key] = build_nc(with_bias=with_bias)
    return _NC_CACHE[key]


def _quant_digits(a, scale):
    """e4m3 leading digit + e4m3 residual digit of a*scale (same scale)."""
    s = (a * scale).astype(np.float32)
    d1 = s.astype(E4NP)
    r1 = (s - d1.astype(np.float32)).astype(E4NP)
    return d1, r1


def _pack_x(xs):
    """[b_loc, IN] fp32 -> [128, nbt, 16, 128] e4m3 (x1 k-tiles then xr1)."""
    b_loc = xs.shape[0]
    nbt = b_loc // P
    ni = xs.shape[1] // P
    x1, xr1 = _quant_digits(xs, SX)
    out = np.empty((P, nbt, 2 * ni, P), E4NP)
    for half, d in ((0, x1), (1, xr1)):
        # d [b_loc, IN] -> T [IN, b_loc] -> [it, ii, bt, bi] -> [ii, bt, kt, bi]
        t = np.ascontiguousarray(d.T).reshape(ni, P, nbt, P)
        out[:, :, half * ni:(half + 1) * ni, :] = t.transpose(1, 2, 0, 3)
    return np.ascontiguousarray(out)


def _pack_w(weight):
    """[K, OUT, IN] fp32 -> [K, 128, 16, OUT] e4m3 (W1 k-tiles then Wr1)."""
    k, out_dim, in_dim = weight.shape
    ni = in_dim // P
    wt = weight.transpose(0, 2, 1).astype(np.float32)  # [K, IN, OUT]
    w1, wr1 = _quant_digits(wt, SW)
    out = np.empty((k, P, 2 * ni, out_dim), E4NP)
    for half, d in ((0, w1), (1, wr1)):
        t = d.reshape(k, ni, P, out_dim)  # [k, it, ii, o]
        out[:, :, half * ni:(half + 1) * ni, :] = t.transpose(0, 2, 1, 3)
    return np.ascontiguousarray(out)


def make_in_maps(x, ew, weight, bias):
    b_loc = B // N_CORES
    nbt = b_loc // P
    wq = _pack_w(weight)
    in_maps = []
    for c in range(N_CORES):
        xs = x[c * b_loc:(c + 1) * b_loc]
        xq = _pack_x(xs)
        ews = ew[c * b_loc:(c + 1) * b_loc]  # [b_loc, K]
        ewp = np.ascontiguousarray(
            ews.reshape(nbt, P, K).transpose(1, 0, 2)
        ) * np.float32(1.0 / (SX * SW))  # [P, nbt, K], descaled
        ewt = np.ascontiguousarray(ews.T)  # [K, b_loc]
        in_maps.append({"xq": xq, "wq": wq, "ewp": ewp, "ewt": ewt, "bias": bias})
    return in_maps


def kernel(x, expert_weights, weight, bias):
    x = np.asarray(x, dtype=np.float32)
    ew = np.asarray(expert_weights, dtype=np.float32)
    weight = np.asarray(weight, dtype=np.float32)
    bias = np.asarray(bias, dtype=np.float32)

    nc = _get_nc(with_bias=bool(np.any(bias)))
    in_maps = make_in_maps(x, ew, weight, bias)
    last_exc = None
    for _attempt in range(3):
        try:
            res = run_bass_kernel_spmd(nc, in_maps, core_ids=list(range(N_CORES)))
            break
        except Exception as exc:  # transient device errors: retry
            last_exc = exc
    else:
        raise last_exc
    y = np.concatenate([r["y"] for r in res.results], axis=0)
    return y


# revision 3
# speedup vs baseline: 1.5473x; 1.5473x over previous
"""ExpertLinear (MoE routing) Trainium2 Bass kernel.

y[b,:] = sum_k ew[b,k] * (x[b,:] @ W[k].T) + (ew @ bias)[b,:]

Strategy: 8-way data-parallel over the batch B across the 8 NeuronCores.
Per core (B_loc = 1024):
  - host supplies layout-prepped shards: xT [IN, B_loc] (x transposed),
    WT [K, IN, OUT] (weights transposed), ewT [K, B_loc], and the
    per-partition routing scalars ewp [128, B_loc/128, K]
  - matmul operands are bf16 (1 col/cycle like fp32r, but ~30% cheaper
    stationary loads and half the HBM traffic; rel err ~2.4e-3 vs the
    2e-2 budget):
        psum[b_tile, :] = sum_i xT[i, b_tile] @ WT[k, i, :]      (per expert k)
        y_acc[b, :]    += ACT(psum * ew[:, k])                   (per-partition scale)
    The second matmul of each (oh0, oh1) PSUM-bank pair reuses the
    stationary operand (ldweights=False).
  - bias term (ewT.T @ bias) is added at the end.
"""

import numpy as np
import ml_dtypes

from concourse import bacc
import concourse.mybir as mybir
import concourse.tile as tile
from concourse.bass_utils import run_bass_kernel_spmd

N_CORES = 8
B, K, OUT, IN = 8192, 8, 1024, 1024
P = 128

MM_DT = mybir.dt.bfloat16
NP_DT = ml_dtypes.bfloat16


def build_nc(b_loc=B // N_CORES, k=K, out_dim=OUT, in_dim=IN, mm_dt=MM_DT, rep=1,
             with_bias=True):
    nbt = b_loc // P      # batch tiles per core
    ni = in_dim // P      # contraction subtiles
    oh_sz = 512           # PSUM bank = 512 fp32
    noh = out_dim // oh_sz

    nc = bacc.Bacc()
    xt_d = nc.dram_tensor("xt", [in_dim, b_loc], mm_dt, kind="ExternalInput")
    wt_d = nc.dram_tensor("wt", [k, in_dim, out_dim], mm_dt, kind="ExternalInput")
    ewp_d = nc.dram_tensor("ewp", [P, nbt, k], mybir.dt.float32, kind="ExternalInput")
    ewt_d = nc.dram_tensor("ewt", [k, b_loc], mm_dt, kind="ExternalInput")
    bias_d = nc.dram_tensor("bias", [k, out_dim], mm_dt, kind="ExternalInput")
    y_d = nc.dram_tensor("y", [b_loc, out_dim], mybir.dt.float32, kind="ExternalOutput")

    with tile.TileContext(nc) as tc:
        with (
            tc.tile_pool(name="consts", bufs=1) as consts,
            tc.tile_pool(name="xt", bufs=1) as xt_pool,
            tc.tile_pool(name="yacc", bufs=1) as yacc_pool,
            tc.tile_pool(name="wbuf", bufs=2) as w_pool,
            tc.tile_pool(name="tmp", bufs=4) as tmp_pool,
            tc.tile_pool(name="ps_mm", bufs=4, space="PSUM") as ps_mm_pool,
        ):
            ewp_sb = consts.tile([P, nbt, k], mybir.dt.float32)
            nc.sync.dma_start(ewp_sb[:], ewp_d[:])
            ewt_sb = consts.tile([k, b_loc], mm_dt)
            nc.sync.dma_start(ewt_sb[:], ewt_d[:])
            bias_sb = consts.tile([k, out_dim], mm_dt)
            nc.sync.dma_start(bias_sb[:], bias_d[:])

            # xT resident, one tile per batch-tile so the first matmuls only
            # wait for their own slice: [128 (i_inner), ni (i_outer), P (b)]
            def load_xt(bt):
                xTbt = xt_pool.tile([P, ni, P], mm_dt, name=f"xT{bt}", tag=f"xT{bt}")
                nc.sync.dma_start(
                    xTbt[:],
                    xt_d[:, bt * P:(bt + 1) * P].rearrange("(io p) b -> p io b", p=P),
                )
                return xTbt

            def load_wchunks(kk):
                # W streamed in per-i chunks so matmuls start as soon as the
                # first contraction slice lands
                wchunks = []
                for i in range(ni):
                    wc = w_pool.tile(
                        [P, out_dim], mm_dt, name=f"wc{i}", tag=f"wc{i}"
                    )
                    nc.sync.dma_start(wc[:], wt_d[kk, i * P:(i + 1) * P, :])
                    wchunks.append(wc)
                return wchunks

            # DMA issue order shapes the critical path: xT[0] and expert-0's
            # W chunks go first so the first matmul series starts as early as
            # possible; the remaining batch tiles follow behind.
            xTs = [None] * nbt
            xTs[0] = load_xt(0)
            wchunks_k0 = load_wchunks(0)
            for bt in range(1, nbt):
                xTs[bt] = load_xt(bt)

            y_acc = yacc_pool.tile([P, nbt, out_dim], mybir.dt.float32)

            for _rep in range(rep):
                # Bias seed: y_acc = ewT.T @ bias. Skipped when the caller
                # knows bias == 0 (expert 0 then writes y_acc directly).
                if with_bias:
                    for bt in range(nbt):
                        pbias = ps_mm_pool.tile(
                            [P, noh, oh_sz], mybir.dt.float32,
                            name="pbias", tag="ps_mm",
                        )
                        for oh in range(noh):
                            nc.tensor.matmul(
                                pbias[:, oh, :],
                                ewt_sb[:, bt * P:(bt + 1) * P],
                                bias_sb[:, oh * oh_sz:(oh + 1) * oh_sz],
                                start=True,
                                stop=True,
                            )
                        for oh in range(noh):
                            nc.scalar.copy(
                                y_acc[:, bt, oh * oh_sz:(oh + 1) * oh_sz],
                                pbias[:, oh, :],
                            )

                # Main loop: stream each expert's WT once; accumulate over
                # the contraction (i) in PSUM, blend over experts (k) into
                # y_acc via ACT per-partition scale + DVE add.
                for kk in range(k):
                    if kk == 0 and _rep == 0:
                        wchunks = wchunks_k0
                    else:
                        wchunks = load_wchunks(kk)
                    for bt in range(nbt):
                        # one PSUM tile spanning both oh banks: the pair's
                        # matmuls share slot state, so the ldweights=False
                        # matmul is always scheduled directly after its
                        # weight-loading partner on the PE queue
                        pss = ps_mm_pool.tile(
                            [P, noh, oh_sz], mybir.dt.float32,
                            name="psmm", tag="ps_mm",
                        )
                        for i in range(ni):
                            lhsT = xTs[bt][:, i, :]
                            for oh in range(noh):
                                nc.tensor.matmul(
                                    pss[:, oh, :],
                                    lhsT,
                                    wchunks[i][:, oh * oh_sz:(oh + 1) * oh_sz],
                                    start=(i == 0),
                                    stop=(i == ni - 1),
                                )
                        for oh in range(noh):
                            osl = y_acc[:, bt, oh * oh_sz:(oh + 1) * oh_sz]
                            scale = ewp_sb[:, bt, kk:kk + 1]
                            if not with_bias and kk == 0:
                                # no bias seed: expert 0 writes y_acc directly
                                nc.scalar.mul(osl, pss[:, oh, :], scale)
                            else:
                                tmp = tmp_pool.tile([P, oh_sz], mybir.dt.float32)
                                nc.scalar.mul(tmp[:], pss[:, oh, :], scale)
                                nc.vector.tensor_add(osl, osl, tmp[:])
                        if kk == k - 1:
                            # y[bt] complete — stream it out while the
                            # remaining batch tiles finish
                            nc.sync.dma_start(
                                y_d[bt * P:(bt + 1) * P, :], y_acc[:, bt, :]
                            )

    nc.compile()

    # Post-compile weight-reuse pass: in the FINAL instruction order, any
    # matmul whose directly-preceding matmul on the PE queue loads the
    # identical stationary AP can skip its reload.
    n_reuse = 0
    for blk in nc.m.functions[0].blocks:
        prev_mm = None
        for inst in blk.instructions:
            if isinstance(inst, mybir.InstMatmult):
                if (
                    prev_mm is not None
                    and not inst.is_transpose
                    and not prev_mm.is_transpose
                    and str(prev_mm.ins[1]) == str(inst.ins[1])
                    and prev_mm.tile_position == inst.tile_position
                ):
                    inst.ldweights = False
                    n_reuse += 1
                prev_mm = inst
    return nc


_NC_CACHE = {}


def _get_nc(with_bias=True):
    key = ("bf16", with_bias)
    if key not in _NC_CACHE:
        _NC_CACHE[key] = build_nc(with_bias=with_bias)
    return _NC_CACHE[key]


def make_in_maps(x, ew, weight, bias):
    b_loc = B // N_CORES
    nbt = b_loc // P
    wt = np.ascontiguousarray(weight.transpose(0, 2, 1)).astype(NP_DT)  # [K, IN, OUT]
    bias16 = bias.astype(NP_DT)
    in_maps = []
    for c in range(N_CORES):
        xs = x[c * b_loc:(c + 1) * b_loc]
        xt = np.ascontiguousarray(xs.T).astype(NP_DT)  # [IN, b_loc]
        ews = ew[c * b_loc:(c + 1) * b_loc]  # [b_loc, K]
        ewp = np.ascontiguousarray(
            ews.reshape(nbt, P, K).transpose(1, 0, 2)
        )  # [P, nbt, K]
        ewt = np.ascontiguousarray(ews.T).astype(NP_DT)  # [K, b_loc]
        in_maps.append({"xt": xt, "wt": wt, "ewp": ewp, "ewt": ewt, "bias": bias16})
    return in_maps


def kernel(x, expert_weights, weight, bias):
    x = np.asarray(x, dtype=np.float32)
    ew = np.asarray(expert_weights, dtype=np.float32)
    weight = np.asarray(weight, dtype=np.float32)
    bias = np.asarray(bias, dtype=np.float32)

    nc = _get_nc(with_bias=bool(np.any(bias)))
    in_maps = make_in_maps(x, ew, weight, bias)
    last_exc = None
    for _attempt in range(3):
        try:
            res = run_bass_kernel_spmd(nc, in_maps, core_ids=list(range(N_CORES)))
            break
        except Exception as exc:  # transient device errors: retry
            last_exc = exc
    else:
        raise last_exc
    y = np.concatenate([r["y"] for r in res.results], axis=0)
    return y
